# revision 9
# baseline (speedup 1.0000x reference)
"""NT-Xent contrastive loss on 8 Trainium2 NeuronCores (Bass/Tile).

Math (matches the reference):
    z  = concat(z_i, z_j)                  [N=8192, D=256] f32
    zn = z / max(||z||_row, 1e-8)
    sim = (zn @ zn.T) / 0.5
    pos[r]  = sim[r, (r+B) mod N]
    lse[r]  = log(sum_{j != r} exp(sim[r, j]))
    loss = mean(lse - pos)

Sharding: rows of z across 8 cores (1024 rows per core).  Every core gets a
copy of z ROLLED by its slab offset, so a single SPMD program works on all
cores: slab rows are always local rows [0, 1024), the self-diagonal block of
M-tile m is at column offset 128*m, and the positive diagonal block at
4096 + 128*m.  Row-wise logsumexp is permutation invariant, so rolling the
column order is harmless.

Per-core kernel:
  * DMA in z (natural, bf16) and zT (pre-transposed on host, bf16, two
    128-partition halves).
  * ss = row sums of squares (DVE tensor_tensor_reduce), then
    r = sqrt(2) * rsqrt(max(ss, 1e-16)) computed as exp(-0.5*ln(ss)+0.5*ln2)
    on ScalarE (exp and ln share one activation table set).  The sqrt(2)
    on each operand folds the 1/T=2 temperature into the GEMM.
  * Scale columns of zT by r (both GEMM operands come from the same zT, so
    one scale pass normalizes rows AND columns of sim).
  * GEMM: per 128-row M-tile, 16 N-chunks of 512 accumulated over K=2x128
    into PSUM groups of [128, 2048]; ScalarE exp with accum_out produces the
    row-sums in the same pass (exp written back in place and discarded).
  * Diagonals (self + positive) extracted pre-exp from PSUM with a single
    DVE tensor_tensor_reduce against an identity matrix.
  * lse = ln(rowsum - exp(self_diag)), contribution = sum(lse - pos),
    reduced across partitions with a ones-vector matmul -> [1,1] output.

Host sums the 8 partial scalars and divides by N.
"""

import math
from contextlib import ExitStack

import numpy as np
import ml_dtypes

import concourse.bass as bass
import concourse.bacc as bacc
import concourse.mybir as mybir
import concourse.tile as tile
from concourse.bass_utils import run_bass_kernel_spmd

P = 128
D = 256
B = 4096
N = 2 * B            # 8192 rows total
NCORES = 8
SLAB = N // NCORES   # 1024 rows per core
MT = SLAB // P       # 8 M-tiles per core
CHUNK = 512          # N-chunk (one PSUM bank at f32)
NCHUNKS = N // CHUNK         # 16
GROUPW = 2048                # ScalarE exp batch = 4 chunks = 4 PSUM banks
NGROUPS = N // GROUPW        # 4
EPS2 = 1e-16                 # max(norm, 1e-8) on the squared norm
HALF_LN2 = 0.5 * math.log(2.0)

F32 = mybir.dt.float32
BF16 = mybir.dt.bfloat16
MULT = mybir.AluOpType.mult
ADD = mybir.AluOpType.add
AF = mybir.ActivationFunctionType


def build_program() -> bass.Bass:
    nc = bacc.Bacc(None, target_bir_lowering=False)

    zt_lo = nc.declare_dram_parameter("zt_lo", [P, N], BF16, isOutput=False)
    zt_hi = nc.declare_dram_parameter("zt_hi", [P, N], BF16, isOutput=False)
    z_nat = nc.declare_dram_parameter("z_nat", [N, D], BF16, isOutput=False)
    ident = nc.declare_dram_parameter("ident", [P, P], F32, isOutput=False)
    partial = nc.declare_dram_parameter("partial", [1, 1], F32, isOutput=True)
    r_dram = nc.dram_tensor("r_vec", [N], BF16)

    with tile.TileContext(nc) as tc:
        with ExitStack() as ctx:
            const = ctx.enter_context(tc.tile_pool(name="const", bufs=1))
            data = ctx.enter_context(tc.tile_pool(name="data", bufs=1))
            stats = ctx.enter_context(tc.tile_pool(name="stats", bufs=1))
            trash = ctx.enter_context(tc.tile_pool(name="trash", bufs=2))
            psum = ctx.enter_context(tc.tile_pool(name="psum", bufs=2, space="PSUM"))

            ident_sb = const.tile([P, P], F32)
            nc.sync.dma_start(out=ident_sb[:], in_=ident[:])
            ones_sb = const.tile([P, 1], F32)
            nc.vector.memset(ones_sb[:], 1.0)
            bias_sb = const.tile([P, 1], F32)
            nc.vector.memset(bias_sb[:], HALF_LN2)

            # ---- natural z (bf16) for row norms: 16 groups of 4 row-tiles
            znat = []
            for g in range(NCHUNKS):
                t = data.tile([P, 4, D], BF16, tag=f"znat{g}")
                src = z_nat[g * CHUNK:(g + 1) * CHUNK, :].rearrange(
                    "(t p) d -> p t d", p=P
                )
                nc.sync.dma_start(out=t[:], in_=src)
                znat.append(t)

            # ---- zT halves, one tile per 512-column chunk
            ztl, zth = [], []
            for i in range(NCHUNKS):
                tl = data.tile([P, CHUNK], BF16, tag=f"ztl{i}")
                nc.sync.dma_start(out=tl[:], in_=zt_lo[:, i * CHUNK:(i + 1) * CHUNK])
                ztl.append(tl)
                th = data.tile([P, CHUNK], BF16, tag=f"zth{i}")
                nc.sync.dma_start(out=th[:], in_=zt_hi[:, i * CHUNK:(i + 1) * CHUNK])
                zth.append(th)

            # ---- ss[p, t] = ||row 128*t + p||^2
            ss_all = stats.tile([P, N // P], F32)
            for t in range(N // P):
                tr = trash.tile([P, D], F32, tag="sqtrash")
                row = znat[t // 4][:, t % 4, :]
                nc.vector.tensor_mul(tr[:], row, row)
                nc.vector.reduce_sum(
                    out=ss_all[:, t:t + 1], in_=tr[:], axis=mybir.AxisListType.X
                )
            nc.vector.tensor_scalar_max(ss_all[:], ss_all[:], EPS2)
            # r = sqrt(2) / sqrt(ss) = exp(-0.5*ln(ss) + 0.5*ln(2))
            nc.scalar.activation(ss_all[:], ss_all[:], AF.Ln)
            r_bf = stats.tile([P, N // P], BF16)
            nc.scalar.activation(
                r_bf[:], ss_all[:], AF.Exp, scale=-0.5, bias=bias_sb[:]
            )

            # round-trip through DRAM to reorder [p, t] -> linear row index,
            # then broadcast across partitions for the column scale
            r_linear = r_dram[:].rearrange("(t p) -> p t", p=P)
            nc.sync.dma_start(out=r_linear, in_=r_bf[:])
            rcol = []
            for i in range(NCHUNKS):
                rc = data.tile([P, CHUNK], BF16, tag=f"rcol{i}")
                nc.gpsimd.dma_start(
                    out=rc[:],
                    in_=r_dram[i * CHUNK:(i + 1) * CHUNK]
                    .rearrange("(a n) -> a n", a=1)
                    .to_broadcast([P, CHUNK]),
                )
                rcol.append(rc)
                nc.vector.tensor_mul(ztl[i][:], ztl[i][:], rc[:])
                nc.vector.tensor_mul(zth[i][:], zth[i][:], rc[:])

            # ---- main GEMM + fused exp/row-sum
            rs4 = stats.tile([P, MT, NGROUPS], F32)
            selfd = stats.tile([P, MT], F32)
            posd = stats.tile([P, MT], F32)
            for m in range(MT):
                ci, co = (m * P) // CHUNK, (m * P) % CHUNK  # lhsT chunk / offset
                for g in range(NGROUPS):
                    ps = psum.tile([P, GROUPW], F32, tag="ps")
                    for c in range(GROUPW // CHUNK):
                        i = g * 4 + c
                        sl = ps[:, c * CHUNK:(c + 1) * CHUNK]
                        nc.tensor.matmul(
                            sl, lhsT=ztl[ci][:, co:co + P], rhs=ztl[i][:],
                            start=True, stop=False,
                        )
                        nc.tensor.matmul(
                            sl, lhsT=zth[ci][:, co:co + P], rhs=zth[i][:],
                            start=False, stop=True,
                        )
                    # self diag sits in group 0, positive diag in group 2,
                    # both at column offset 128*m within their group
                    if g == 0 or g == 2:
                        acc = selfd if g == 0 else posd
                        tr = trash.tile([P, P], F32, tag="dtrash")
                        nc.vector.tensor_mul(
                            tr[:], ps[:, m * P:(m + 1) * P], ident_sb[:]
                        )
                        nc.vector.reduce_sum(
                            out=acc[:, m:m + 1], in_=tr[:],
                            axis=mybir.AxisListType.X,
                        )
                    nc.scalar.activation(
                        ps[:], ps[:], AF.Exp, accum_out=rs4[:, m, g:g + 1]
                    )

            # ---- tail: lse = ln(rowsum - exp(self)), contrib = sum(lse - pos)
            rs = stats.tile([P, MT], F32)
            nc.vector.reduce_sum(out=rs[:], in_=rs4[:], axis=mybir.AxisListType.X)
            eself = stats.tile([P, MT], F32)
            nc.scalar.activation(eself[:], selfd[:], AF.Exp)
            nc.vector.tensor_sub(rs[:], rs[:], eself[:])
            nc.scalar.activation(rs[:], rs[:], AF.Ln)
            nc.vector.tensor_sub(rs[:], rs[:], posd[:])
            contrib = stats.tile([P, 1], F32)
            nc.vector.reduce_sum(out=contrib[:], in_=rs[:], axis=mybir.AxisListType.X)

            psf = psum.tile([P, GROUPW], F32, tag="ps")
            nc.tensor.matmul(
                psf[0:1, 0:1], lhsT=contrib[:], rhs=ones_sb[:], start=True, stop=True
            )
            out_sb = stats.tile([1, 1], F32)
            nc.vector.tensor_copy(out_sb[:], psf[0:1, 0:1])
            nc.sync.dma_start(out=partial[:], in_=out_sb[:])

    nc.compile()
    return nc


_PROGRAM = None


def _get_program() -> bass.Bass:
    global _PROGRAM
    if _PROGRAM is None:
        _PROGRAM = build_program()
    return _PROGRAM


def make_in_maps(z_i: np.ndarray, z_j: np.ndarray) -> list[dict]:
    z = np.concatenate(
        [np.asarray(z_i, dtype=np.float32), np.asarray(z_j, dtype=np.float32)], axis=0
    )
    zb = z.astype(ml_dtypes.bfloat16)          # [N, D]
    zt = np.ascontiguousarray(zb.T)            # [D, N]
    ident = np.eye(P, dtype=np.float32)
    in_maps = []
    for c in range(NCORES):
        sh = SLAB * c
        zr = np.ascontiguousarray(np.roll(zb, -sh, axis=0))
        ztr = np.roll(zt, -sh, axis=1)
        in_maps.append({
            "zt_lo": np.ascontiguousarray(ztr[:P]),
            "zt_hi": np.ascontiguousarray(ztr[P:]),
            "z_nat": zr,
            "ident": ident,
        })
    return in_maps


def kernel_with_results(z_i: np.ndarray, z_j: np.ndarray, trace: bool = False):
    nc = _get_program()
    in_maps = make_in_maps(z_i, z_j)
    res = run_bass_kernel_spmd(nc, in_maps, list(range(NCORES)), trace=trace)
    total = sum(float(r["partial"][0, 0]) for r in res.results)
    return np.float32(total / N), res


def kernel(z_i: np.ndarray, z_j: np.ndarray) -> np.ndarray:
    out, _ = kernel_with_results(z_i, z_j)
    return out


# revision 10
# speedup vs baseline: 1.3049x; 1.3049x over previous
"""NT-Xent contrastive loss on 8 Trainium2 NeuronCores (Bass/Tile).

Math (matches the reference):
    z  = concat(z_i, z_j)                  [N=8192, D=256] f32
    zn = z / max(||z||_row, 1e-8)
    sim = (zn @ zn.T) / 0.5
    pos[r]  = sim[r, (r+B) mod N]
    lse[r]  = log(sum_{j != r} exp(sim[r, j]))
    loss = mean(lse - pos)

Sharding: rows of z across 8 cores (1024 rows per core).  Every core gets a
copy of z ROLLED by its slab offset, so a single SPMD program works on all
cores: slab rows are always local rows [0, 1024), the self-diagonal block of
M-tile m is at column offset 128*m, and the positive diagonal block at
4096 + 128*m.  Row-wise logsumexp is permutation invariant, so rolling the
column order is harmless.

Per-core kernel:
  * DMA in z twice, bf16: natural layout packed 64-rows-per-partition (for
    row norms, fully contiguous 32KB/partition lines) and pre-transposed
    zT in two 128-partition halves (host does the transpose).
  * ss = row sums of squares (batched DVE mul+reduce), then
    r = sqrt(2) * rsqrt(max(ss, 1e-16)) computed as exp(-0.5*ln(ss)+0.5*ln2)
    on ScalarE (exp and ln share one activation table set).  The sqrt(2)
    on each operand folds the 1/T=2 temperature into the GEMM.
  * r round-trips through DRAM on the SWDGE queue (not the busy HWDGE load
    queue) to reorder into linear row order, then partition-broadcasts.
  * Scale columns of zT by r (both GEMM operands come from the same zT, so
    one scale pass normalizes rows AND columns of sim).
  * GEMM: per 128-row M-tile, 16 N-chunks of 512 accumulated over K=2x128
    into PSUM groups of [128, 2048]; ScalarE exp with accum_out produces the
    row-sums in the same pass (exp written back in place and discarded).
  * Diagonals (self + positive) extracted pre-exp from PSUM with DVE
    mul-by-identity + row-reduce.
  * lse = ln(rowsum - exp(self_diag)), contribution = sum(lse - pos),
    reduced across partitions with a ones-vector matmul -> [1,1] output.

Host sums the 8 partial scalars and divides by N.
"""

import math
from contextlib import ExitStack

import numpy as np
import ml_dtypes

import concourse.bass as bass
import concourse.bacc as bacc
import concourse.mybir as mybir
import concourse.tile as tile
from concourse.bass_utils import run_bass_kernel_spmd

P = 128
D = 256
B = 4096
N = 2 * B            # 8192 rows total
NCORES = 8
SLAB = N // NCORES   # 1024 rows per core
MT = SLAB // P       # 8 M-tiles per core
CHUNK = 512          # matmul moving-operand width (one PSUM bank at f32)
GROUPW = 2048        # ScalarE exp batch = 4 chunks = 4 PSUM banks
NGROUPS = N // GROUPW        # 4
RPP = N // P                 # rows per partition in packed natural layout (64)
EPS2 = 1e-16                 # max(norm, 1e-8) on the squared norm
HALF_LN2 = 0.5 * math.log(2.0)

F32 = mybir.dt.float32
BF16 = mybir.dt.bfloat16
AF = mybir.ActivationFunctionType
AX = mybir.AxisListType


def build_program() -> bass.Bass:
    nc = bacc.Bacc(None, target_bir_lowering=False)

    zt_lo = nc.declare_dram_parameter("zt_lo", [P, N], BF16, isOutput=False)
    zt_hi = nc.declare_dram_parameter("zt_hi", [P, N], BF16, isOutput=False)
    # natural z, packed: partition p holds rows [64p, 64p+64), contiguous
    z_nat = nc.declare_dram_parameter("z_nat", [N, D], BF16, isOutput=False)
    ident = nc.declare_dram_parameter("ident", [P, P], F32, isOutput=False)
    partial = nc.declare_dram_parameter("partial", [1, 1], F32, isOutput=True)
    r_dram = nc.dram_tensor("r_vec", [N], BF16)

    with tile.TileContext(nc) as tc:
        with ExitStack() as ctx:
            const = ctx.enter_context(tc.tile_pool(name="const", bufs=1))
            data = ctx.enter_context(tc.tile_pool(name="data", bufs=1))
            stats = ctx.enter_context(tc.tile_pool(name="stats", bufs=1))
            trash = ctx.enter_context(tc.tile_pool(name="trash", bufs=2))
            psum = ctx.enter_context(tc.tile_pool(name="psum", bufs=2, space="PSUM"))

            ident_sb = const.tile([P, P], F32)
            nc.scalar.dma_start(out=ident_sb[:], in_=ident[:])
            ones_sb = const.tile([P, 1], F32)
            nc.vector.memset(ones_sb[:], 1.0)
            bias_sb = const.tile([P, 1], F32)
            nc.vector.memset(bias_sb[:], HALF_LN2)

            # ---- natural z for norms: [128, 64, 256], partition p = rows
            # 64p..64p+64.  4 DMAs of 8KB-contiguous per-partition lines.
            znat = data.tile([P, RPP, D], BF16)
            zsrc = z_nat[:].rearrange("(p t) d -> p t d", p=P)
            NDMA = 4
            step = RPP // NDMA  # 16 row-blocks per DMA
            for c in range(NDMA):
                nc.sync.dma_start(
                    out=znat[:, c * step:(c + 1) * step, :],
                    in_=zsrc[:, c * step:(c + 1) * step, :],
                )

            # ---- zT halves, one tile per 2048-column group (4KB/partition)
            ztl, zth = [], []
            for g in range(NGROUPS):
                tl = data.tile([P, GROUPW], BF16, tag=f"ztl{g}")
                nc.sync.dma_start(out=tl[:], in_=zt_lo[:, g * GROUPW:(g + 1) * GROUPW])
                ztl.append(tl)
                th = data.tile([P, GROUPW], BF16, tag=f"zth{g}")
                nc.scalar.dma_start(out=th[:], in_=zt_hi[:, g * GROUPW:(g + 1) * GROUPW])
                zth.append(th)

            # ---- ss[p, t] = ||row 64p + t||^2, batched mul+reduce
            ss_all = stats.tile([P, RPP], F32)
            for c in range(NDMA):
                blk = znat[:, c * step:(c + 1) * step, :]
                tr = trash.tile([P, step, D], BF16, tag="sqtrash")
                nc.vector.tensor_mul(tr[:], blk, blk)
                nc.vector.reduce_sum(
                    out=ss_all[:, c * step:(c + 1) * step], in_=tr[:], axis=AX.X
                )
            nc.vector.tensor_scalar_max(ss_all[:], ss_all[:], EPS2)
            # r = sqrt(2) / sqrt(ss) = exp(-0.5*ln(ss) + 0.5*ln(2))
            nc.scalar.activation(ss_all[:], ss_all[:], AF.Ln)
            r_bf = stats.tile([P, RPP], BF16)
            nc.scalar.activation(
                r_bf[:], ss_all[:], AF.Exp, scale=-0.5, bias=bias_sb[:]
            )

            # round-trip through DRAM (SWDGE — skips the loaded HWDGE queue)
            # to reorder [p, t] -> linear row index 64p+t, then broadcast
            r_linear = r_dram[:].rearrange("(p t) -> p t", p=P)
            nc.gpsimd.dma_start(out=r_linear, in_=r_bf[:])
            rcol = []
            for g in range(NGROUPS):
                rc = data.tile([P, GROUPW], BF16, tag=f"rcol{g}")
                nc.gpsimd.dma_start(
                    out=rc[:],
                    in_=r_dram[g * GROUPW:(g + 1) * GROUPW]
                    .rearrange("(a n) -> a n", a=1)
                    .to_broadcast([P, GROUPW]),
                )
                rcol.append(rc)
                nc.vector.tensor_mul(ztl[g][:], ztl[g][:], rc[:])
                nc.vector.tensor_mul(zth[g][:], zth[g][:], rc[:])

            # ---- main GEMM + fused exp/row-sum
            rs4 = stats.tile([P, MT, NGROUPS], F32)
            selfd = stats.tile([P, MT], F32)
            posd = stats.tile([P, MT], F32)
            for m in range(MT):
                lo_l = ztl[0][:, m * P:(m + 1) * P]   # lhsT slices (cols < 1024)
                lo_h = zth[0][:, m * P:(m + 1) * P]
                for g in range(NGROUPS):
                    ps = psum.tile([P, GROUPW], F32, tag="ps")
                    # k-outer order: one LDWEIGHTS per k-half per group
                    for c in range(GROUPW // CHUNK):
                        nc.tensor.matmul(
                            ps[:, c * CHUNK:(c + 1) * CHUNK],
                            lhsT=lo_l,
                            rhs=ztl[g][:, c * CHUNK:(c + 1) * CHUNK],
                            start=True, stop=False,
                        )
                    for c in range(GROUPW // CHUNK):
                        nc.tensor.matmul(
                            ps[:, c * CHUNK:(c + 1) * CHUNK],
                            lhsT=lo_h,
                            rhs=zth[g][:, c * CHUNK:(c + 1) * CHUNK],
                            start=False, stop=True,
                        )
                    # self diag sits in group 0, positive diag in group 2,
                    # both at column offset 128*m within their group
                    if g == 0 or g == 2:
                        acc = selfd if g == 0 else posd
                        tr = trash.tile([P, P], F32, tag="dtrash")
                        nc.vector.tensor_mul(
                            tr[:], ps[:, m * P:(m + 1) * P], ident_sb[:]
                        )
                        nc.vector.reduce_sum(
                            out=acc[:, m:m + 1], in_=tr[:], axis=AX.X
                        )
                    nc.scalar.activation(
                        ps[:], ps[:], AF.Exp, accum_out=rs4[:, m, g:g + 1]
                    )

            # ---- tail: lse = ln(rowsum - exp(self)), contrib = sum(lse - pos)
            rs = stats.tile([P, MT], F32)
            nc.vector.reduce_sum(out=rs[:], in_=rs4[:], axis=AX.X)
            eself = stats.tile([P, MT], F32)
            nc.scalar.activation(eself[:], selfd[:], AF.Exp)
            nc.vector.tensor_sub(rs[:], rs[:], eself[:])
            nc.scalar.activation(rs[:], rs[:], AF.Ln)
            nc.vector.tensor_sub(rs[:], rs[:], posd[:])
            contrib = stats.tile([P, 1], F32)
            nc.vector.reduce_sum(out=contrib[:], in_=rs[:], axis=AX.X)

            psf = psum.tile([P, GROUPW], F32, tag="ps")
            nc.tensor.matmul(
                psf[0:1, 0:1], lhsT=contrib[:], rhs=ones_sb[:], start=True, stop=True
            )
            out_sb = stats.tile([1, 1], F32)
            nc.vector.tensor_copy(out_sb[:], psf[0:1, 0:1])
            nc.sync.dma_start(out=partial[:], in_=out_sb[:])

    nc.compile()
    return nc


_PROGRAM = None


def _get_program() -> bass.Bass:
    global _PROGRAM
    if _PROGRAM is None:
        _PROGRAM = build_program()
    return _PROGRAM


def make_in_maps(z_i: np.ndarray, z_j: np.ndarray) -> list[dict]:
    z = np.concatenate(
        [np.asarray(z_i, dtype=np.float32), np.asarray(z_j, dtype=np.float32)], axis=0
    )
    zb = z.astype(ml_dtypes.bfloat16)          # [N, D]
    zt = np.ascontiguousarray(zb.T)            # [D, N]
    ident = np.eye(P, dtype=np.float32)
    in_maps = []
    for c in range(NCORES):
        sh = SLAB * c
        zr = np.ascontiguousarray(np.roll(zb, -sh, axis=0))
        ztr = np.roll(zt, -sh, axis=1)
        in_maps.append({
            "zt_lo": np.ascontiguousarray(ztr[:P]),
            "zt_hi": np.ascontiguousarray(ztr[P:]),
            "z_nat": zr,
            "ident": ident,
        })
    return in_maps


def kernel_with_results(z_i: np.ndarray, z_j: np.ndarray, trace: bool = False):
    nc = _get_program()
    in_maps = make_in_maps(z_i, z_j)
    res = run_bass_kernel_spmd(nc, in_maps, list(range(NCORES)), trace=trace)
    total = sum(float(r["partial"][0, 0]) for r in res.results)
    return np.float32(total / N), res


def kernel(z_i: np.ndarray, z_j: np.ndarray) -> np.ndarray:
    out, _ = kernel_with_results(z_i, z_j)
    return out


# revision 11
# speedup vs baseline: 1.3444x; 1.0302x over previous
"""NT-Xent contrastive loss on 8 Trainium2 NeuronCores (Bass/Tile).

Math (matches the reference):
    z  = concat(z_i, z_j)                  [N=8192, D=256] f32
    zn = z / max(||z||_row, 1e-8)
    sim = (zn @ zn.T) / 0.5
    pos[r]  = sim[r, (r+B) mod N]
    lse[r]  = log(sum_{j != r} exp(sim[r, j]))
    loss = mean(lse - pos)

Sharding: rows of z across 8 cores (1024 rows per core).  Every core gets a
copy of z ROLLED by its slab offset, so a single SPMD program works on all
cores: slab rows are always local rows [0, 1024), the self-diagonal block of
M-tile m is at column offset 128*m, and the positive diagonal block at
4096 + 128*m.  Row-wise logsumexp is permutation invariant, so rolling the
column order is harmless.

Per-core kernel:
  * DMA in z twice, bf16: natural layout packed 64-rows-per-partition (for
    row norms, fully contiguous 32KB/partition lines) and pre-transposed
    zT in two 128-partition halves (host does the transpose).
  * ss = row sums of squares (batched DVE mul+reduce), then
    r = sqrt(2) * rsqrt(max(ss, 1e-16)) computed as exp(-0.5*ln(ss)+0.5*ln2)
    on ScalarE (exp and ln share one activation table set).  The sqrt(2)
    on each operand folds the 1/T=2 temperature into the GEMM.
  * r round-trips through DRAM on the SWDGE queue (not the busy HWDGE load
    queue) to reorder into linear row order, then partition-broadcasts.
  * Scale columns of zT by r (both GEMM operands come from the same zT, so
    one scale pass normalizes rows AND columns of sim).
  * GEMM: per 128-row M-tile, 16 N-chunks of 512 accumulated over K=2x128
    into PSUM groups of [128, 2048]; ScalarE exp with accum_out produces the
    row-sums in the same pass (exp written back in place and discarded).
  * Diagonals (self + positive) extracted pre-exp from PSUM with DVE
    mul-by-identity + row-reduce.
  * lse = ln(rowsum - exp(self_diag)), contribution = sum(lse - pos),
    reduced across partitions with a ones-vector matmul -> [1,1] output.

Host sums the 8 partial scalars and divides by N.
"""

import math
from contextlib import ExitStack

import numpy as np
import ml_dtypes

import concourse.bass as bass
import concourse.bacc as bacc
import concourse.mybir as mybir
import concourse.tile as tile
from concourse.bass_utils import run_bass_kernel_spmd

P = 128
D = 256
B = 4096
N = 2 * B            # 8192 rows total
NCORES = 8
SLAB = N // NCORES   # 1024 rows per core
MT = SLAB // P       # 8 M-tiles per core
CHUNK = 512          # matmul moving-operand width (one PSUM bank at f32)
GROUPW = 2048        # ScalarE exp batch = 4 chunks = 4 PSUM banks
NGROUPS = N // GROUPW        # 4
RPP = N // P                 # rows per partition in packed natural layout (64)
EPS2 = 1e-16                 # max(norm, 1e-8) on the squared norm
HALF_LN2 = 0.5 * math.log(2.0)

F32 = mybir.dt.float32
BF16 = mybir.dt.bfloat16
AF = mybir.ActivationFunctionType
AX = mybir.AxisListType


def build_program() -> bass.Bass:
    nc = bacc.Bacc(None, target_bir_lowering=False)

    zt_lo = nc.declare_dram_parameter("zt_lo", [P, N], BF16, isOutput=False)
    zt_hi = nc.declare_dram_parameter("zt_hi", [P, N], BF16, isOutput=False)
    # natural z, packed: partition p holds rows [64p, 64p+64), contiguous
    z_nat = nc.declare_dram_parameter("z_nat", [N, D], BF16, isOutput=False)
    ident = nc.declare_dram_parameter("ident", [P, P], F32, isOutput=False)
    partial = nc.declare_dram_parameter("partial", [1, 1], F32, isOutput=True)
    r_dram = nc.dram_tensor("r_vec", [N], BF16)

    with tile.TileContext(nc) as tc:
        with ExitStack() as ctx:
            const = ctx.enter_context(tc.tile_pool(name="const", bufs=1))
            data = ctx.enter_context(tc.tile_pool(name="data", bufs=1))
            stats = ctx.enter_context(tc.tile_pool(name="stats", bufs=1))
            trash = ctx.enter_context(tc.tile_pool(name="trash", bufs=2))
            psum = ctx.enter_context(tc.tile_pool(name="psum", bufs=2, space="PSUM"))

            ident_sb = const.tile([P, P], F32)
            nc.scalar.dma_start(out=ident_sb[:], in_=ident[:])
            ones_sb = const.tile([P, 1], F32)
            nc.vector.memset(ones_sb[:], 1.0)
            bias_sb = const.tile([P, 1], F32)
            nc.vector.memset(bias_sb[:], HALF_LN2)

            # ---- per-group pipeline: each 2048-row/column group g flows
            # DMA -> norms -> r -> broadcast -> scale independently, so the
            # GEMM can start as soon as group 0 is ready.
            RB = GROUPW // P  # 16 rows per partition per group block
            znat_g, ztl, zth = [], [], []
            for g in range(NGROUPS):
                zn = data.tile([P, RB, D], BF16, tag=f"znat{g}")
                src = z_nat[g * GROUPW:(g + 1) * GROUPW, :].rearrange(
                    "(p t) d -> p t d", p=P
                )
                nc.sync.dma_start(out=zn[:], in_=src)
                znat_g.append(zn)
            for g in range(NGROUPS):
                tl = data.tile([P, GROUPW], BF16, tag=f"ztl{g}")
                nc.sync.dma_start(out=tl[:], in_=zt_lo[:, g * GROUPW:(g + 1) * GROUPW])
                ztl.append(tl)
                th = data.tile([P, GROUPW], BF16, tag=f"zth{g}")
                nc.scalar.dma_start(out=th[:], in_=zt_hi[:, g * GROUPW:(g + 1) * GROUPW])
                zth.append(th)

            rcol = []
            for g in range(NGROUPS):
                # ss[p, t] = ||row 2048g + 16p + t||^2
                blk = znat_g[g]
                tr = trash.tile([P, RB, D], BF16, tag="sqtrash")
                nc.vector.tensor_mul(tr[:], blk[:], blk[:])
                ss_g = stats.tile([P, RB], F32, tag=f"ss{g}")
                nc.vector.reduce_sum(out=ss_g[:], in_=tr[:], axis=AX.X)
                nc.vector.tensor_scalar_max(ss_g[:], ss_g[:], EPS2)
                # r = sqrt(2)/sqrt(ss) = exp(-0.5*ln(ss) + 0.5*ln(2))
                nc.scalar.activation(ss_g[:], ss_g[:], AF.Ln)
                r_g = stats.tile([P, RB], BF16, tag=f"r{g}")
                nc.scalar.activation(
                    r_g[:], ss_g[:], AF.Exp, scale=-0.5, bias=bias_sb[:]
                )
                # DRAM round-trip on SWDGE to linearize [p, t] -> 16p + t,
                # then partition-broadcast for the column scale
                r_lin = (
                    r_dram[g * GROUPW:(g + 1) * GROUPW]
                    .rearrange("(p t) -> p t", p=P)
                )
                nc.gpsimd.dma_start(out=r_lin, in_=r_g[:])
                rc = data.tile([P, GROUPW], BF16, tag=f"rcol{g}")
                nc.gpsimd.dma_start(
                    out=rc[:],
                    in_=r_dram[g * GROUPW:(g + 1) * GROUPW]
                    .rearrange("(a n) -> a n", a=1)
                    .to_broadcast([P, GROUPW]),
                )
                rcol.append(rc)
                nc.vector.tensor_mul(ztl[g][:], ztl[g][:], rc[:])
                nc.vector.tensor_mul(zth[g][:], zth[g][:], rc[:])

            # ---- main GEMM + fused exp/row-sum (group-major: group 0 work
            # starts while later groups are still being normalized)
            rs4 = stats.tile([P, MT, NGROUPS], F32)
            selfd = stats.tile([P, MT], F32)
            posd = stats.tile([P, MT], F32)
            for g in range(NGROUPS):
                for m in range(MT):
                    lo_l = ztl[0][:, m * P:(m + 1) * P]  # lhsT slices (cols < 1024)
                    lo_h = zth[0][:, m * P:(m + 1) * P]
                    ps = psum.tile([P, GROUPW], F32, tag="ps")
                    # k-outer order: weights shared across the 4 chunks
                    for c in range(GROUPW // CHUNK):
                        nc.tensor.matmul(
                            ps[:, c * CHUNK:(c + 1) * CHUNK],
                            lhsT=lo_l,
                            rhs=ztl[g][:, c * CHUNK:(c + 1) * CHUNK],
                            start=True, stop=False,
                        )
                    for c in range(GROUPW // CHUNK):
                        nc.tensor.matmul(
                            ps[:, c * CHUNK:(c + 1) * CHUNK],
                            lhsT=lo_h,
                            rhs=zth[g][:, c * CHUNK:(c + 1) * CHUNK],
                            start=False, stop=True,
                        )
                    # self diag sits in group 0, positive diag in group 2,
                    # both at column offset 128*m within their group
                    if g == 0 or g == 2:
                        acc = selfd if g == 0 else posd
                        tr = trash.tile([P, P], F32, tag="dtrash")
                        nc.vector.tensor_mul(
                            tr[:], ps[:, m * P:(m + 1) * P], ident_sb[:]
                        )
                        nc.vector.reduce_sum(
                            out=acc[:, m:m + 1], in_=tr[:], axis=AX.X
                        )
                    nc.scalar.activation(
                        ps[:], ps[:], AF.Exp, accum_out=rs4[:, m, g:g + 1]
                    )

            # ---- tail: lse = ln(rowsum - exp(self)), contrib = sum(lse - pos)
            rs = stats.tile([P, MT], F32)
            nc.vector.reduce_sum(out=rs[:], in_=rs4[:], axis=AX.X)
            eself = stats.tile([P, MT], F32)
            nc.scalar.activation(eself[:], selfd[:], AF.Exp)
            nc.vector.tensor_sub(rs[:], rs[:], eself[:])
            nc.scalar.activation(rs[:], rs[:], AF.Ln)
            nc.vector.tensor_sub(rs[:], rs[:], posd[:])
            contrib = stats.tile([P, 1], F32)
            nc.vector.reduce_sum(out=contrib[:], in_=rs[:], axis=AX.X)

            psf = psum.tile([P, GROUPW], F32, tag="ps")
            nc.tensor.matmul(
                psf[0:1, 0:1], lhsT=contrib[:], rhs=ones_sb[:], start=True, stop=True
            )
            out_sb = stats.tile([1, 1], F32)
            nc.vector.tensor_copy(out_sb[:], psf[0:1, 0:1])
            nc.sync.dma_start(out=partial[:], in_=out_sb[:])

    nc.compile()
    return nc


_PROGRAM = None


def _get_program() -> bass.Bass:
    global _PROGRAM
    if _PROGRAM is None:
        _PROGRAM = build_program()
    return _PROGRAM


def make_in_maps(z_i: np.ndarray, z_j: np.ndarray) -> list[dict]:
    z = np.concatenate(
        [np.asarray(z_i, dtype=np.float32), np.asarray(z_j, dtype=np.float32)], axis=0
    )
    zb = z.astype(ml_dtypes.bfloat16)          # [N, D]
    zt = np.ascontiguousarray(zb.T)            # [D, N]
    ident = np.eye(P, dtype=np.float32)
    in_maps = []
    for c in range(NCORES):
        sh = SLAB * c
        zr = np.ascontiguousarray(np.roll(zb, -sh, axis=0))
        ztr = np.roll(zt, -sh, axis=1)
        in_maps.append({
            "zt_lo": np.ascontiguousarray(ztr[:P]),
            "zt_hi": np.ascontiguousarray(ztr[P:]),
            "z_nat": zr,
            "ident": ident,
        })
    return in_maps


def kernel_with_results(z_i: np.ndarray, z_j: np.ndarray, trace: bool = False):
    nc = _get_program()
    in_maps = make_in_maps(z_i, z_j)
    res = run_bass_kernel_spmd(nc, in_maps, list(range(NCORES)), trace=trace)
    total = sum(float(r["partial"][0, 0]) for r in res.results)
    return np.float32(total / N), res


def kernel(z_i: np.ndarray, z_j: np.ndarray) -> np.ndarray:
    out, _ = kernel_with_results(z_i, z_j)
    return out
